# revision 59
# baseline (speedup 1.0000x reference)
"""STConvBlock Trainium2 kernel: tconv1(GLU) -> Cheb-attention -> tconv2(GLU) -> norm.

Sharding: 40 (slice, head) attention units, 5 per core (unit u = core*5 + j);
one AllGather of all unit outputs, then the replicated tail (tconv2+norm,
16 slices) runs from a ring cache of head-averaged slices. All adjacency
masks live in SBUF (fp8); all hot matmuls/DVE ops in bf16.

Attention math (per slice xs [N,C], head h, chebyshev k):
  wx = xs @ Wt;  al = xs @ (Wt@Wl.T);  ar = xs @ (Wt@Wr.T)   # [N,3]
  S[i,j] = sum_r m_r[i,j] * (al[i,r] + ar[j,r]); masked softmax_j; out += p @ wx
We build S TRANSPOSED (S^T[j,i]): ar[j,r] is a per-partition scalar, al is
broadcast via a K=1 PE matmul into PSUM, and each relation term is ONE fused
DVE scalar_tensor_tensor (al_bcast + ar) * mask. E^T = U*exp(S^T) feeds the
PE as rhs; lhsT=[wx|ones] yields p@wx (unscaled) and the softmax denominators
in one PSUM tile.
"""

import numpy as np
import ml_dtypes

B, T, N, C = 2, 12, 1024, 64
KT = 3
T1 = T - KT + 1   # 10
T2 = T1 - KT + 1  # 8
H, K1, R = 2, 3, 2
NSLICE = B * T1   # 20
NUNITS = NSLICE * H  # 40
N_CORES = 8
NT = N // 128  # 8
FCH = 512
NF = N // FCH  # 2
NC_ELEMS = float(N * C)
NTAIL = B * T2  # 16
# tail slices runnable after round j's AllGather (needs att slices <= 4j+3;
# tail g=(b,t2) needs slices b*T1+t2 .. +2)
TAIL_WINDOWS = [[0, 1], [2, 3, 4, 5], [6, 7], [8, 9, 10, 11], [12, 13, 14, 15]]
AVR = 6  # ring depth of cached head-averaged slices

_cache = {}


def _build(n_cores, do_att=True, do_tail=True, debug=False):
    import concourse.bass as bass
    import concourse.tile as tile
    import concourse.mybir as mybir
    from concourse import bacc
    from concourse.masks import make_identity

    F32 = mybir.dt.float32
    BF16 = mybir.dt.bfloat16
    FP8 = mybir.dt.float8e4
    AF = mybir.ActivationFunctionType
    ALU = mybir.AluOpType
    AX = mybir.AxisListType
    upc = NUNITS // n_cores

    nc = bacc.Bacc(None, target_bir_lowering=False)
    xw = nc.dram_tensor("xw", [upc, KT, C, N], BF16, kind="ExternalInput")
    w1T = nc.dram_tensor("w1T", [KT, C, 2 * C], BF16, kind="ExternalInput")
    w2T = nc.dram_tensor("w2T", [KT, C, 2 * C], BF16, kind="ExternalInput")
    # combined per-(unit,k) weights: cols [wt (C) | wr (3) | wl (3)]
    wcU = nc.dram_tensor("wcU", [upc, K1, C, C + 6], BF16, kind="ExternalInput")
    mrelT = nc.dram_tensor("mrelT", [R, N, N], FP8, kind="ExternalInput")
    msupT = nc.dram_tensor("msupT", [K1, N, N], FP8, kind="ExternalInput")
    uT = nc.dram_tensor("uT", [K1, N, N], FP8, kind="ExternalInput")
    gbT = nc.dram_tensor("gbT", [2, C, N], F32, kind="ExternalInput")
    out = nc.dram_tensor("out", [B, T2, C, N], F32, kind="ExternalOutput")
    dbg = (
        nc.dram_tensor("dbg", [14, 128, N], BF16, kind="ExternalOutput")
        if debug else None
    )
    ag_inA = nc.dram_tensor("ag_inA", [3, C, N], BF16)
    ag_inB = nc.dram_tensor("ag_inB", [2, C, N], BF16)
    agA = nc.dram_tensor("agA", [3 * n_cores, C, N], BF16, addr_space="Shared")
    agB = nc.dram_tensor("agB", [2 * n_cores, C, N], BF16, addr_space="Shared")

    with tile.TileContext(nc) as tc:
        with (
            tc.tile_pool(name="consts", bufs=1) as consts,
            tc.tile_pool(name="work", bufs=2) as work,
            tc.tile_pool(name="sc", bufs=2) as sc,
            tc.tile_pool(name="ps_al", bufs=1, space="PSUM") as ps_al,
            tc.tile_pool(name="ps_m", bufs=2, space="PSUM") as ps_m,
            tc.tile_pool(name="ps_big", bufs=1, space="PSUM") as ps_big,
            tc.tile_pool(name="ps_glu", bufs=1, space="PSUM") as ps_glu,
        ):
            # ---- resident constants ----
            mrel_sb = consts.tile([128, R, NT, N], FP8)
            nc.sync.dma_start(
                out=mrel_sb[:],
                in_=mrelT[:].rearrange("r (t p) n -> p r t n", p=128),
            )
            msup_sb = consts.tile([128, K1, NT, N], FP8)
            nc.sync.dma_start(
                out=msup_sb[:],
                in_=msupT[:].rearrange("k (t p) n -> p k t n", p=128),
            )
            w1_sb = consts.tile([C, KT, 2 * C], BF16)
            w2_sb = consts.tile([C, KT, 2 * C], BF16)
            nc.sync.dma_start(out=w1_sb[:], in_=w1T[:].rearrange("t c o -> c t o"))
            nc.sync.dma_start(out=w2_sb[:], in_=w2T[:].rearrange("t c o -> c t o"))
            wc_sb = consts.tile([C, upc, K1, C + 6], BF16)
            nc.sync.dma_start(out=wc_sb[:], in_=wcU[:].rearrange("j k c x -> c j k x"))
            gb_sb = consts.tile([C, 2, N], F32)
            nc.sync.dma_start(out=gb_sb[:, 0, :], in_=gbT[0])
            nc.sync.dma_start(out=gb_sb[:, 1, :], in_=gbT[1])
            ones1x128 = consts.tile([1, 128], BF16)
            nc.gpsimd.memset(ones1x128, 1.0)
            ones1x64 = consts.tile([1, C], F32)
            nc.gpsimd.memset(ones1x64, 1.0)
            ones64x1 = consts.tile([C, 1], F32)
            nc.gpsimd.memset(ones64x1, 1.0)
            id64 = consts.tile([C, C], F32)
            make_identity(nc, id64)
            eps_sb = consts.tile([1, 1], F32)
            nc.gpsimd.memset(eps_sb, 1e-6)
            # head-averaged attention outputs, ring-cached for the tail
            avs = consts.tile([C, AVR, N], BF16)

            def glu_conv(w_sb, rhs_tile, res_cb, out_tile, res_scale):
                """out[c, n] = (conv[0:C] + res_scale*res) * sigmoid(conv[C:2C])."""
                for f in range(NF):
                    cps = ps_glu.tile([2 * C, FCH], F32, tag="glu")
                    for tau in range(KT):
                        nc.tensor.matmul(
                            out=cps,
                            lhsT=w_sb[:, tau, :],
                            rhs=rhs_tile(tau)[:, f * FCH : (f + 1) * FCH],
                            start=(tau == 0),
                            stop=(tau == KT - 1),
                        )
                    sig = work.tile([C, FCH], BF16, tag="sig")
                    nc.scalar.activation(out=sig, in_=cps[C:, :], func=AF.Sigmoid)
                    po = work.tile([C, FCH], BF16, tag="po")
                    nc.vector.affine_then_add(
                        out=po, in0=res_cb(f), in1=cps[:C, :],
                        scale=res_scale, bias=0.0,
                    )
                    nc.vector.tensor_mul(
                        out=out_tile[:, f * FCH : (f + 1) * FCH], in0=po, in1=sig
                    )

            def tail_slice(g):
                b, t2 = divmod(g, T2)
                s0 = b * T1 + t2
                h2 = work.tile([C, N], F32, tag="h2", bufs=2)
                # w2 pre-scaled by 0.5 on host (head-mean fold); residual 0.5
                glu_conv(
                    w2_sb,
                    lambda tau: avs[:, (s0 + tau) % AVR, :],
                    lambda f: avs[:, (s0 + KT - 1) % AVR, f * FCH : (f + 1) * FCH],
                    h2,
                    0.5,
                )
                sums = work.tile([C, 1], F32, tag="sums")
                nc.vector.tensor_reduce(out=sums, in_=h2, axis=AX.X, op=ALU.add)
                sqs = work.tile([C, 1], F32, tag="sqs")
                scr = work.tile([C, N], F32, tag="scr", bufs=1)
                nc.scalar.activation(out=scr, in_=h2, func=AF.Square)
                nc.vector.tensor_reduce(out=sqs, in_=scr, axis=AX.X, op=ALU.add)
                pair = work.tile([C, 2], F32, tag="pair")
                nc.scalar.copy(out=pair[:, 0:1], in_=sums)
                nc.scalar.copy(out=pair[:, 1:2], in_=sqs)
                totp = ps_m.tile([1, 2], F32, tag="m")
                nc.tensor.matmul(
                    out=totp, lhsT=ones64x1, rhs=pair, start=True, stop=True
                )
                mu = work.tile([1, 1], F32, tag="mu")
                nc.scalar.mul(mu, totp[:, 0:1], 1.0 / NC_ELEMS)
                nmusq = work.tile([1, 1], F32, tag="nmusq")
                nc.vector.tensor_mul(out=nmusq, in0=mu, in1=mu)
                nc.scalar.mul(nmusq, nmusq, -1.0)
                var = work.tile([1, 1], F32, tag="var")
                nc.scalar.activation(
                    out=var, in_=totp[:, 1:2],
                    func=AF.Identity, scale=1.0 / NC_ELEMS, bias=nmusq,
                )
                sd = work.tile([1, 1], F32, tag="sd")
                nc.scalar.activation(out=sd, in_=var, func=AF.Sqrt, bias=eps_sb)
                rstd = work.tile([1, 1], F32, tag="rstd")
                nc.vector.reciprocal(out=rstd, in_=sd)
                nmr = work.tile([1, 1], F32, tag="nmr")
                nc.vector.tensor_mul(out=nmr, in0=mu, in1=rstd)
                nc.scalar.mul(nmr, nmr, -1.0)
                pr = work.tile([1, 2], F32, tag="pr")
                nc.scalar.copy(out=pr[:, 0:1], in_=rstd)
                nc.scalar.copy(out=pr[:, 1:2], in_=nmr)
                bcp = ps_m.tile([C, 2], F32, tag="m")
                nc.tensor.matmul(
                    out=bcp, lhsT=ones1x64, rhs=pr, start=True, stop=True
                )
                bc = work.tile([C, 2], F32, tag="bc")
                nc.scalar.copy(out=bc, in_=bcp)
                normed = work.tile([C, N], F32, tag="normed", bufs=1)
                nc.scalar.activation(
                    out=normed, in_=h2, func=AF.Identity,
                    scale=bc[:, 0:1], bias=bc[:, 1:2],
                )
                og = work.tile([C, N], F32, tag="og", bufs=1)
                nc.vector.tensor_mul(out=og, in0=normed, in1=gb_sb[:, 0, :])
                nc.vector.tensor_add(out=og, in0=og, in1=gb_sb[:, 1, :])
                nc.sync.dma_start(out=out[b, t2], in_=og)

            # ---- attention units (5 rounds, pipelined with tail) ----
            for j in range(upc if do_att else 0):
                xw_t = work.tile([C, KT, N], BF16, tag="xw", bufs=1)
                for tau in range(KT):
                    nc.sync.dma_start(out=xw_t[:, tau, :], in_=xw[j, tau])
                xsT = work.tile([C, N], BF16, tag="xsT")
                glu_conv(
                    w1_sb,
                    lambda tau: xw_t[:, tau, :],
                    lambda f: xw_t[:, KT - 1, f * FCH : (f + 1) * FCH],
                    xsT,
                    1.0,
                )
                accT = work.tile([C, N], BF16, tag="accT", bufs=1)
                for k in range(K1):
                    # per jt: one matmul -> [wx | ar] ; plus al rows
                    ars, wxos = [], []
                    for jt in range(NT):
                        cps = ps_m.tile([128, C + 3], F32, tag="m")
                        nc.tensor.matmul(
                            out=cps, lhsT=xsT[:, jt * 128 : (jt + 1) * 128],
                            rhs=wc_sb[:, j, k, : C + 3], start=True, stop=True,
                        )
                        ar_jt = work.tile([128, R + 1], F32, tag=f"ar{jt}", bufs=2)
                        nc.scalar.copy(out=ar_jt, in_=cps[:, C : C + 3])
                        ars.append(ar_jt)
                        wxo_jt = work.tile([128, C + 1], BF16, tag=f"wxo{jt}", bufs=2)
                        nc.gpsimd.memset(wxo_jt[:, C : C + 1], 1.0)
                        nc.scalar.copy(out=wxo_jt[:, :C], in_=cps[:, :C])
                        wxos.append(wxo_jt)
                    alTs = []
                    for r in range(R + 1):
                        alT_r = work.tile([1, N], BF16, tag=f"alT{r}", bufs=2)
                        alTs.append(alT_r)
                    for f in range(NF):
                        for r in range(R + 1):
                            alp = ps_m.tile([1, FCH], F32, tag="m")
                            nc.tensor.matmul(
                                out=alp,
                                lhsT=wc_sb[:, j, k, C + 3 + r : C + 4 + r],
                                rhs=xsT[:, f * FCH : (f + 1) * FCH],
                                start=True, stop=True,
                            )
                            nc.scalar.copy(
                                out=alTs[r][:, f * FCH : (f + 1) * FCH], in_=alp
                            )
                    # al broadcast -> SBUF bf16 [128, 3, N] (one ACT copy per f)
                    als_sb = work.tile([128, R + 1, N], BF16, tag="als")
                    for f in range(NF):
                        alp = ps_al.tile([128, R + 1, FCH], F32, tag="al")
                        for r in range(R + 1):
                            nc.tensor.matmul(
                                out=alp[:, r, :], lhsT=ones1x128,
                                rhs=alTs[r][:, f * FCH : (f + 1) * FCH],
                                start=True, stop=True,
                            )
                        nc.scalar.copy(
                            out=als_sb[:, :, f * FCH : (f + 1) * FCH], in_=alp
                        )
                    # union mask for this k: one batched DMA
                    uuk = sc.tile([128, NT, N], FP8, tag="uuk", bufs=1)
                    nc.sync.dma_start(
                        out=uuk, in_=uT[k].rearrange("(t p) n -> p t n", p=128)
                    )
                    opsf = []
                    for f in range(NF):
                        ops_f = ps_big.tile([C + 1, FCH], F32, tag=f"big{f}")
                        opsf.append(ops_f)
                    for jt in range(NT):
                        t0 = sc.tile([128, N], BF16, tag="t0")
                        nc.vector.scalar_tensor_tensor(
                            out=t0, in0=als_sb[:, 0, :], scalar=ars[jt][:, 0:1],
                            in1=mrel_sb[:, 0, jt, :],
                            op0=ALU.add, op1=ALU.mult,
                        )
                        t1 = sc.tile([128, N], BF16, tag="t1")
                        nc.vector.scalar_tensor_tensor(
                            out=t1, in0=als_sb[:, 1, :], scalar=ars[jt][:, 1:2],
                            in1=mrel_sb[:, 1, jt, :],
                            op0=ALU.add, op1=ALU.mult,
                        )
                        s01 = sc.tile([128, N], BF16, tag="s01")
                        nc.vector.tensor_add(out=s01, in0=t0, in1=t1)
                        t2 = sc.tile([128, N], BF16, tag="t2")
                        nc.vector.scalar_tensor_tensor(
                            out=t2, in0=als_sb[:, 2, :], scalar=ars[jt][:, 2:3],
                            in1=msup_sb[:, k, jt, :],
                            op0=ALU.add, op1=ALU.mult,
                        )
                        ss = sc.tile([128, N], BF16, tag="ss")
                        nc.vector.tensor_add(out=ss, in0=s01, in1=t2)
                        eraw = sc.tile([128, N], BF16, tag="eraw")
                        nc.scalar.activation(out=eraw, in_=ss, func=AF.Exp)
                        ee = sc.tile([128, N], BF16, tag="ee")
                        nc.vector.tensor_mul(out=ee, in0=eraw, in1=uuk[:, jt, :])
                        if debug and j == 0 and k == 0 and jt == 0:
                            nc.sync.dma_start(out=dbg[1], in_=als_sb[:, 0, :])
                            nc.sync.dma_start(out=dbg[2], in_=ee)
                            nc.sync.dma_start(out=dbg[6], in_=ss)
                            nc.sync.dma_start(out=dbg[7], in_=eraw)
                        for f in range(NF):
                            nc.tensor.matmul(
                                out=opsf[f], lhsT=wxos[jt],
                                rhs=ee[:, f * FCH : (f + 1) * FCH],
                                start=(jt == 0), stop=(jt == NT - 1),
                            )
                    for f in range(NF):
                        fs = slice(f * FCH, (f + 1) * FCH)
                        ops = opsf[f]
                        recip = work.tile([1, FCH], F32, tag="recip")
                        nc.vector.reciprocal(out=recip, in_=ops[C : C + 1, :])
                        rbp = ps_m.tile([C, FCH], F32, tag="m")
                        nc.tensor.matmul(
                            out=rbp, lhsT=ones1x64, rhs=recip, start=True, stop=True
                        )
                        rb_sb = work.tile([C, FCH], BF16, tag="rb")
                        nc.scalar.copy(out=rb_sb, in_=rbp)
                        pk = work.tile([C, FCH], BF16, tag="pk")
                        nc.scalar.copy(out=pk, in_=ops[:C, :])
                        if debug and j == 0 and k == 0 and f == 0:
                            nc.sync.dma_start(out=dbg[0, :C], in_=xsT)
                            nc.sync.dma_start(out=dbg[3, :C, :FCH], in_=pk)
                            nc.sync.dma_start(out=dbg[4, :C, :FCH], in_=rb_sb)
                            den = work.tile([1, FCH], BF16, tag="dbgden")
                            nc.scalar.copy(out=den, in_=ops[C : C + 1, :])
                            nc.sync.dma_start(out=dbg[5, 0:1, :FCH], in_=den)
                        if k == 0:
                            nc.vector.tensor_mul(
                                out=accT[:, fs], in0=pk, in1=rb_sb,
                            )
                        else:
                            tsc = work.tile([C, FCH], BF16, tag="tsc")
                            nc.vector.tensor_mul(out=tsc, in0=pk, in1=rb_sb)
                            nc.vector.tensor_add(
                                out=accT[:, fs], in0=accT[:, fs], in1=tsc,
                            )
                # elu(accT) = exp(min(a,0)) - 1 + relu(a)
                mn = work.tile([C, N], BF16, tag="mn", bufs=1)
                nc.vector.tensor_scalar_min(mn, accT, 0.0)
                ex = work.tile([C, N], BF16, tag="ex", bufs=1)
                nc.scalar.activation(out=ex, in_=mn, func=AF.Exp)
                rl = work.tile([C, N], BF16, tag="rl", bufs=1)
                nc.scalar.activation(out=rl, in_=accT, func=AF.Relu)
                elu = work.tile([C, N], BF16, tag="elu")
                nc.vector.affine_then_add(out=elu, in0=ex, in1=rl, scale=1.0, bias=-1.0)
                if j < 3:
                    nc.sync.dma_start(out=ag_inA[j], in_=elu)
                else:
                    nc.sync.dma_start(out=ag_inB[j - 3], in_=elu)

                # interleaved unit mapping u = j*8 + core: units 0..23 are
                # done after round 2 -> gather them and run the first 8 tail
                # slices overlapped with rounds 3-4; rest after round 4.
                def gathered(u):
                    if u < 24:
                        return agA[(u % n_cores) * 3 + u // n_cores]
                    return agB[(u % n_cores) * 2 + u // n_cores - 3]

                if j == 2 or j == 4:
                    nc.gpsimd.collective_compute(
                        "AllGather",
                        mybir.AluOpType.bypass,
                        replica_groups=[list(range(n_cores))],
                        ins=[ag_inA[:] if j == 2 else ag_inB[:]],
                        outs=[agA[:] if j == 2 else agB[:]],
                    )
                    if do_tail:
                        srange = range(0, 12) if j == 2 else range(12, NSLICE)
                        for s in srange:
                            a0 = work.tile([C, N], BF16, tag="ga0", bufs=2)
                            nc.sync.dma_start(out=a0, in_=gathered(2 * s))
                            a1 = work.tile([C, N], BF16, tag="ga1", bufs=2)
                            nc.sync.dma_start(out=a1, in_=gathered(2 * s + 1))
                            nc.vector.tensor_add(
                                out=avs[:, s % AVR, :], in0=a0, in1=a1
                            )
                            # run every tail slice whose inputs are now all
                            # cached; ring depth AVR covers the 3-slice window
                            for g in range(NTAIL):
                                b, t2 = divmod(g, T2)
                                if b * T1 + t2 + KT - 1 == s:
                                    tail_slice(g)

    if not nc.is_finalized():
        nc.finalize()
    return nc


def _prep(inputs, n_cores):
    x = np.asarray(inputs["x"], np.float32)
    supports = np.asarray(inputs["supports"], np.float32)
    atten = np.asarray(inputs["atten_supports"], np.float32)
    w_t1 = np.asarray(inputs["w_t1"], np.float32)
    Wt = np.asarray(inputs["Wt"], np.float32)
    Wl = np.asarray(inputs["Wl"], np.float32)
    Wr = np.asarray(inputs["Wr"], np.float32)
    w_t2 = np.asarray(inputs["w_t2"], np.float32)
    gamma = np.asarray(inputs["gamma"], np.float32)
    beta = np.asarray(inputs["beta"], np.float32)

    fp8 = ml_dtypes.float8_e4m3fn
    bf16 = ml_dtypes.bfloat16
    xT = np.ascontiguousarray(x.transpose(0, 1, 3, 2)).astype(bf16)  # [B,T,C,N]
    w1T = np.ascontiguousarray(w_t1[:, :, :, 0].transpose(2, 1, 0)).astype(bf16)
    # conv(w2, 0.5*(h0+h1)) = conv(0.5*w2, h0+h1): head-mean folded into w2
    w2T = np.ascontiguousarray(
        (0.5 * w_t2[:, :, :, 0]).transpose(2, 1, 0)
    ).astype(bf16)
    mrel = (atten != 0).astype(np.float32)  # [R,N,N]
    msup = (supports != 0).astype(np.float32)  # [K1,N,N]
    mrelT = np.ascontiguousarray(mrel.transpose(0, 2, 1)).astype(fp8)
    msupT = np.ascontiguousarray(msup.transpose(0, 2, 1)).astype(fp8)
    uT = np.ascontiguousarray(
        np.minimum(mrel[0] + mrel[1] + msup, 1.0).transpose(0, 2, 1)
    ).astype(fp8)
    gbT = np.stack([gamma[0, 0].T, beta[0, 0].T]).astype(np.float32)  # [2,C,N]

    upc = NUNITS // n_cores
    in_maps = []
    for c in range(n_cores):
        xwa = np.empty((upc, KT, C, N), bf16)
        wcU = np.empty((upc, K1, C, C + 6), bf16)
        for j in range(upc):
            u = j * n_cores + c  # interleaved: rounds 0-2 cover units 0-23
            sl, h = u // H, u % H
            b, t1 = sl // T1, sl % T1
            for tau in range(KT):
                xwa[j, tau] = xT[b, t1 + tau]
            for k in range(K1):
                wcU[j, k, :, :C] = Wt[h, k]
                wcU[j, k, :, C : C + 3] = Wt[h, k] @ Wr[h, k].T
                wcU[j, k, :, C + 3 :] = Wt[h, k] @ Wl[h, k].T
        in_maps.append(
            dict(xw=xwa, w1T=w1T, w2T=w2T, wcU=wcU,
                 mrelT=mrelT, msupT=msupT, uT=uT, gbT=gbT)
        )
    return in_maps


def kernel(**inputs):
    from concourse.bass_utils import run_bass_kernel_spmd

    if N_CORES not in _cache:
        _cache[N_CORES] = _build(N_CORES)
    nc = _cache[N_CORES]
    in_maps = _prep(inputs, N_CORES)
    res = run_bass_kernel_spmd(nc, in_maps, list(range(N_CORES)))
    o = np.asarray(res.results[0]["out"], np.float32)  # [B,T2,C,N]
    return np.ascontiguousarray(o.transpose(0, 1, 3, 2))


# revision 61
# speedup vs baseline: 1.2183x; 1.2183x over previous
"""STConvBlock Trainium2 kernel: tconv1(GLU) -> Cheb-attention -> tconv2(GLU) -> norm.

Sharding: 40 (slice, head) attention units, 5 per core (unit u = core*5 + j);
one AllGather of all unit outputs, then the replicated tail (tconv2+norm,
16 slices) runs from a ring cache of head-averaged slices. All adjacency
masks live in SBUF (fp8); all hot matmuls/DVE ops in bf16.

Attention math (per slice xs [N,C], head h, chebyshev k):
  wx = xs @ Wt;  al = xs @ (Wt@Wl.T);  ar = xs @ (Wt@Wr.T)   # [N,3]
  S[i,j] = sum_r m_r[i,j] * (al[i,r] + ar[j,r]); masked softmax_j; out += p @ wx
We build S TRANSPOSED (S^T[j,i]): ar[j,r] is a per-partition scalar, al is
broadcast via a K=1 PE matmul into PSUM, and each relation term is ONE fused
DVE scalar_tensor_tensor (al_bcast + ar) * mask. E^T = U*exp(S^T) feeds the
PE as rhs; lhsT=[wx|ones] yields p@wx (unscaled) and the softmax denominators
in one PSUM tile.
"""

import numpy as np
import ml_dtypes

B, T, N, C = 2, 12, 1024, 64
KT = 3
T1 = T - KT + 1   # 10
T2 = T1 - KT + 1  # 8
H, K1, R = 2, 3, 2
NSLICE = B * T1   # 20
NUNITS = NSLICE * H  # 40
N_CORES = 8
NT = N // 128  # 8
FCH = 512
NF = N // FCH  # 2
NC_ELEMS = float(N * C)
NTAIL = B * T2  # 16
# tail slices runnable after round j's AllGather (needs att slices <= 4j+3;
# tail g=(b,t2) needs slices b*T1+t2 .. +2)
TAIL_WINDOWS = [[0, 1], [2, 3, 4, 5], [6, 7], [8, 9, 10, 11], [12, 13, 14, 15]]
AVR = 6  # ring depth of cached head-averaged slices

_cache = {}


def _build(n_cores, do_att=True, do_tail=True, debug=False):
    import concourse.bass as bass
    import concourse.tile as tile
    import concourse.mybir as mybir
    from concourse import bacc
    from concourse.masks import make_identity

    F32 = mybir.dt.float32
    BF16 = mybir.dt.bfloat16
    FP8 = mybir.dt.float8e4
    AF = mybir.ActivationFunctionType
    ALU = mybir.AluOpType
    AX = mybir.AxisListType
    upc = NUNITS // n_cores

    nc = bacc.Bacc(None, target_bir_lowering=False)
    xw = nc.dram_tensor("xw", [upc, KT, C, N], BF16, kind="ExternalInput")
    w1T = nc.dram_tensor("w1T", [KT, C, 2 * C], BF16, kind="ExternalInput")
    w2T = nc.dram_tensor("w2T", [KT, C, 2 * C], BF16, kind="ExternalInput")
    # combined per-(unit,k) weights: cols [wt (C) | wr (3) | wl (3)]
    wcU = nc.dram_tensor("wcU", [upc, K1, C, C + 6], BF16, kind="ExternalInput")
    mrelT = nc.dram_tensor("mrelT", [R, N, N], FP8, kind="ExternalInput")
    msupT = nc.dram_tensor("msupT", [K1, N, N], FP8, kind="ExternalInput")
    uT = nc.dram_tensor("uT", [K1, N, N], FP8, kind="ExternalInput")
    gbT = nc.dram_tensor("gbT", [2, C, N], F32, kind="ExternalInput")
    out = nc.dram_tensor("out", [B, T2, C, N], F32, kind="ExternalOutput")
    dbg = (
        nc.dram_tensor("dbg", [14, 128, N], BF16, kind="ExternalOutput")
        if debug else None
    )
    ag_in = nc.dram_tensor("ag_in", [upc, C, N], BF16)
    ag_out = nc.dram_tensor("ag_out", [NUNITS, C, N], BF16, addr_space="Shared")

    with tile.TileContext(nc) as tc:
        with (
            tc.tile_pool(name="consts", bufs=1) as consts,
            tc.tile_pool(name="work", bufs=2) as work,
            tc.tile_pool(name="sc", bufs=2) as sc,
            tc.tile_pool(name="ps_al", bufs=1, space="PSUM") as ps_al,
            tc.tile_pool(name="ps_m", bufs=2, space="PSUM") as ps_m,
            tc.tile_pool(name="ps_big", bufs=1, space="PSUM") as ps_big,
            tc.tile_pool(name="ps_glu", bufs=1, space="PSUM") as ps_glu,
        ):
            # ---- resident constants ----
            mrel_sb = consts.tile([128, R, NT, N], FP8)
            nc.sync.dma_start(
                out=mrel_sb[:],
                in_=mrelT[:].rearrange("r (t p) n -> p r t n", p=128),
            )
            msup_sb = consts.tile([128, K1, NT, N], FP8)
            nc.sync.dma_start(
                out=msup_sb[:],
                in_=msupT[:].rearrange("k (t p) n -> p k t n", p=128),
            )
            w1_sb = consts.tile([C, KT, 2 * C], BF16)
            w2_sb = consts.tile([C, KT, 2 * C], BF16)
            nc.sync.dma_start(out=w1_sb[:], in_=w1T[:].rearrange("t c o -> c t o"))
            nc.sync.dma_start(out=w2_sb[:], in_=w2T[:].rearrange("t c o -> c t o"))
            wc_sb = consts.tile([C, upc, K1, C + 6], BF16)
            nc.sync.dma_start(out=wc_sb[:], in_=wcU[:].rearrange("j k c x -> c j k x"))
            gb_sb = consts.tile([C, 2, N], F32)
            nc.sync.dma_start(out=gb_sb[:, 0, :], in_=gbT[0])
            nc.sync.dma_start(out=gb_sb[:, 1, :], in_=gbT[1])
            ones1x128 = consts.tile([1, 128], BF16)
            nc.gpsimd.memset(ones1x128, 1.0)
            ones1x64 = consts.tile([1, C], F32)
            nc.gpsimd.memset(ones1x64, 1.0)
            ones64x1 = consts.tile([C, 1], F32)
            nc.gpsimd.memset(ones64x1, 1.0)
            id64 = consts.tile([C, C], F32)
            make_identity(nc, id64)
            eps_sb = consts.tile([1, 1], F32)
            nc.gpsimd.memset(eps_sb, 1e-6)
            # head-averaged attention outputs, ring-cached for the tail
            avs = consts.tile([C, AVR, N], BF16)

            def glu_conv(w_sb, rhs_tile, res_cb, out_tile, res_scale):
                """out[c, n] = (conv[0:C] + res_scale*res) * sigmoid(conv[C:2C])."""
                for f in range(NF):
                    cps = ps_glu.tile([2 * C, FCH], F32, tag="glu")
                    for tau in range(KT):
                        nc.tensor.matmul(
                            out=cps,
                            lhsT=w_sb[:, tau, :],
                            rhs=rhs_tile(tau)[:, f * FCH : (f + 1) * FCH],
                            start=(tau == 0),
                            stop=(tau == KT - 1),
                        )
                    sig = work.tile([C, FCH], BF16, tag="sig")
                    nc.scalar.activation(out=sig, in_=cps[C:, :], func=AF.Sigmoid)
                    po = work.tile([C, FCH], BF16, tag="po")
                    nc.vector.affine_then_add(
                        out=po, in0=res_cb(f), in1=cps[:C, :],
                        scale=res_scale, bias=0.0,
                    )
                    nc.vector.tensor_mul(
                        out=out_tile[:, f * FCH : (f + 1) * FCH], in0=po, in1=sig
                    )

            def tail_slice(g):
                b, t2 = divmod(g, T2)
                s0 = b * T1 + t2
                h2 = work.tile([C, N], F32, tag="h2", bufs=2)
                # w2 pre-scaled by 0.5 on host (head-mean fold); residual 0.5
                glu_conv(
                    w2_sb,
                    lambda tau: avs[:, (s0 + tau) % AVR, :],
                    lambda f: avs[:, (s0 + KT - 1) % AVR, f * FCH : (f + 1) * FCH],
                    h2,
                    0.5,
                )
                sums = work.tile([C, 1], F32, tag="sums")
                nc.vector.tensor_reduce(out=sums, in_=h2, axis=AX.X, op=ALU.add)
                sqs = work.tile([C, 1], F32, tag="sqs")
                scr = work.tile([C, N], F32, tag="scr", bufs=1)
                nc.scalar.activation(out=scr, in_=h2, func=AF.Square)
                nc.vector.tensor_reduce(out=sqs, in_=scr, axis=AX.X, op=ALU.add)
                pair = work.tile([C, 2], F32, tag="pair")
                nc.scalar.copy(out=pair[:, 0:1], in_=sums)
                nc.scalar.copy(out=pair[:, 1:2], in_=sqs)
                totp = ps_m.tile([1, 2], F32, tag="m")
                nc.tensor.matmul(
                    out=totp, lhsT=ones64x1, rhs=pair, start=True, stop=True
                )
                mu = work.tile([1, 1], F32, tag="mu")
                nc.scalar.mul(mu, totp[:, 0:1], 1.0 / NC_ELEMS)
                nmusq = work.tile([1, 1], F32, tag="nmusq")
                nc.vector.tensor_mul(out=nmusq, in0=mu, in1=mu)
                nc.scalar.mul(nmusq, nmusq, -1.0)
                var = work.tile([1, 1], F32, tag="var")
                nc.scalar.activation(
                    out=var, in_=totp[:, 1:2],
                    func=AF.Identity, scale=1.0 / NC_ELEMS, bias=nmusq,
                )
                sd = work.tile([1, 1], F32, tag="sd")
                nc.scalar.activation(out=sd, in_=var, func=AF.Sqrt, bias=eps_sb)
                rstd = work.tile([1, 1], F32, tag="rstd")
                nc.vector.reciprocal(out=rstd, in_=sd)
                nmr = work.tile([1, 1], F32, tag="nmr")
                nc.vector.tensor_mul(out=nmr, in0=mu, in1=rstd)
                nc.scalar.mul(nmr, nmr, -1.0)
                pr = work.tile([1, 2], F32, tag="pr")
                nc.scalar.copy(out=pr[:, 0:1], in_=rstd)
                nc.scalar.copy(out=pr[:, 1:2], in_=nmr)
                bcp = ps_m.tile([C, 2], F32, tag="m")
                nc.tensor.matmul(
                    out=bcp, lhsT=ones1x64, rhs=pr, start=True, stop=True
                )
                bc = work.tile([C, 2], F32, tag="bc")
                nc.scalar.copy(out=bc, in_=bcp)
                normed = work.tile([C, N], F32, tag="normed", bufs=1)
                nc.scalar.activation(
                    out=normed, in_=h2, func=AF.Identity,
                    scale=bc[:, 0:1], bias=bc[:, 1:2],
                )
                og = work.tile([C, N], F32, tag="og", bufs=1)
                nc.vector.tensor_mul(out=og, in0=normed, in1=gb_sb[:, 0, :])
                nc.vector.tensor_add(out=og, in0=og, in1=gb_sb[:, 1, :])
                nc.sync.dma_start(out=out[b, t2], in_=og)

            # ---- attention units (5 rounds, pipelined with tail) ----
            for j in range(upc if do_att else 0):
                xw_t = work.tile([C, KT, N], BF16, tag="xw", bufs=1)
                for tau in range(KT):
                    nc.sync.dma_start(out=xw_t[:, tau, :], in_=xw[j, tau])
                xsT = work.tile([C, N], BF16, tag="xsT")
                glu_conv(
                    w1_sb,
                    lambda tau: xw_t[:, tau, :],
                    lambda f: xw_t[:, KT - 1, f * FCH : (f + 1) * FCH],
                    xsT,
                    1.0,
                )
                accT = work.tile([C, N], BF16, tag="accT", bufs=1)
                for k in range(K1):
                    # per jt: one matmul -> [wx | ar] ; plus al rows
                    ars, wxos = [], []
                    for jt in range(NT):
                        cps = ps_m.tile([128, C + 3], F32, tag="m")
                        nc.tensor.matmul(
                            out=cps, lhsT=xsT[:, jt * 128 : (jt + 1) * 128],
                            rhs=wc_sb[:, j, k, : C + 3], start=True, stop=True,
                        )
                        ar_jt = work.tile([128, R + 1], F32, tag=f"ar{jt}", bufs=2)
                        nc.scalar.copy(out=ar_jt, in_=cps[:, C : C + 3])
                        ars.append(ar_jt)
                        wxo_jt = work.tile([128, C + 1], BF16, tag=f"wxo{jt}", bufs=2)
                        nc.gpsimd.memset(wxo_jt[:, C : C + 1], 1.0)
                        nc.scalar.copy(out=wxo_jt[:, :C], in_=cps[:, :C])
                        wxos.append(wxo_jt)
                    alTs = []
                    for r in range(R + 1):
                        alT_r = work.tile([1, N], BF16, tag=f"alT{r}", bufs=2)
                        alTs.append(alT_r)
                    for f in range(NF):
                        for r in range(R + 1):
                            alp = ps_m.tile([1, FCH], F32, tag="m")
                            nc.tensor.matmul(
                                out=alp,
                                lhsT=wc_sb[:, j, k, C + 3 + r : C + 4 + r],
                                rhs=xsT[:, f * FCH : (f + 1) * FCH],
                                start=True, stop=True,
                            )
                            nc.scalar.copy(
                                out=alTs[r][:, f * FCH : (f + 1) * FCH], in_=alp
                            )
                    # al broadcast -> SBUF bf16 [128, 3, N] (one ACT copy per f)
                    als_sb = work.tile([128, R + 1, N], BF16, tag="als")
                    for f in range(NF):
                        alp = ps_al.tile([128, R + 1, FCH], F32, tag="al")
                        for r in range(R + 1):
                            nc.tensor.matmul(
                                out=alp[:, r, :], lhsT=ones1x128,
                                rhs=alTs[r][:, f * FCH : (f + 1) * FCH],
                                start=True, stop=True,
                            )
                        nc.scalar.copy(
                            out=als_sb[:, :, f * FCH : (f + 1) * FCH], in_=alp
                        )
                    # union mask for this k: one batched DMA
                    uuk = sc.tile([128, NT, N], FP8, tag="uuk", bufs=1)
                    nc.sync.dma_start(
                        out=uuk, in_=uT[k].rearrange("(t p) n -> p t n", p=128)
                    )
                    opsf = []
                    for f in range(NF):
                        ops_f = ps_big.tile([C + 1, FCH], F32, tag=f"big{f}")
                        opsf.append(ops_f)
                    def emit_ee(eraw_p, jt_p):
                        # ee + its E-matmuls, deferred one tile so the DVE's
                        # strict FIFO never head-of-line blocks on ACT's exp
                        ee = sc.tile([128, N], BF16, tag="ee")
                        nc.vector.tensor_mul(
                            out=ee, in0=eraw_p, in1=uuk[:, jt_p, :]
                        )
                        for f in range(NF):
                            nc.tensor.matmul(
                                out=opsf[f], lhsT=wxos[jt_p],
                                rhs=ee[:, f * FCH : (f + 1) * FCH],
                                start=(jt_p == 0), stop=(jt_p == NT - 1),
                            )

                    pend = None
                    for jt in range(NT):
                        t0 = sc.tile([128, N], BF16, tag="t0")
                        nc.vector.scalar_tensor_tensor(
                            out=t0, in0=als_sb[:, 0, :], scalar=ars[jt][:, 0:1],
                            in1=mrel_sb[:, 0, jt, :],
                            op0=ALU.add, op1=ALU.mult,
                        )
                        t1 = sc.tile([128, N], BF16, tag="t1")
                        nc.vector.scalar_tensor_tensor(
                            out=t1, in0=als_sb[:, 1, :], scalar=ars[jt][:, 1:2],
                            in1=mrel_sb[:, 1, jt, :],
                            op0=ALU.add, op1=ALU.mult,
                        )
                        s01 = sc.tile([128, N], BF16, tag="s01")
                        nc.vector.tensor_add(out=s01, in0=t0, in1=t1)
                        t2 = sc.tile([128, N], BF16, tag="t2")
                        nc.vector.scalar_tensor_tensor(
                            out=t2, in0=als_sb[:, 2, :], scalar=ars[jt][:, 2:3],
                            in1=msup_sb[:, k, jt, :],
                            op0=ALU.add, op1=ALU.mult,
                        )
                        ss = sc.tile([128, N], BF16, tag="ss")
                        nc.vector.tensor_add(out=ss, in0=s01, in1=t2)
                        eraw = sc.tile([128, N], BF16, tag="eraw")
                        nc.scalar.activation(out=eraw, in_=ss, func=AF.Exp)
                        if pend is not None:
                            emit_ee(*pend)
                        pend = (eraw, jt)
                    emit_ee(*pend)
                    for f in range(NF):
                        fs = slice(f * FCH, (f + 1) * FCH)
                        ops = opsf[f]
                        recip = work.tile([1, FCH], F32, tag="recip")
                        nc.vector.reciprocal(out=recip, in_=ops[C : C + 1, :])
                        rbp = ps_m.tile([C, FCH], F32, tag="m")
                        nc.tensor.matmul(
                            out=rbp, lhsT=ones1x64, rhs=recip, start=True, stop=True
                        )
                        rb_sb = work.tile([C, FCH], BF16, tag="rb")
                        nc.scalar.copy(out=rb_sb, in_=rbp)
                        pk = work.tile([C, FCH], BF16, tag="pk")
                        nc.scalar.copy(out=pk, in_=ops[:C, :])
                        if debug and j == 0 and k == 0 and f == 0:
                            nc.sync.dma_start(out=dbg[0, :C], in_=xsT)
                            nc.sync.dma_start(out=dbg[3, :C, :FCH], in_=pk)
                            nc.sync.dma_start(out=dbg[4, :C, :FCH], in_=rb_sb)
                            den = work.tile([1, FCH], BF16, tag="dbgden")
                            nc.scalar.copy(out=den, in_=ops[C : C + 1, :])
                            nc.sync.dma_start(out=dbg[5, 0:1, :FCH], in_=den)
                        if k == 0:
                            nc.vector.tensor_mul(
                                out=accT[:, fs], in0=pk, in1=rb_sb,
                            )
                        else:
                            tsc = work.tile([C, FCH], BF16, tag="tsc")
                            nc.vector.tensor_mul(out=tsc, in0=pk, in1=rb_sb)
                            nc.vector.tensor_add(
                                out=accT[:, fs], in0=accT[:, fs], in1=tsc,
                            )
                # elu(accT) = exp(min(a,0)) - 1 + relu(a)
                mn = work.tile([C, N], BF16, tag="mn", bufs=1)
                nc.vector.tensor_scalar_min(mn, accT, 0.0)
                ex = work.tile([C, N], BF16, tag="ex", bufs=1)
                nc.scalar.activation(out=ex, in_=mn, func=AF.Exp)
                rl = work.tile([C, N], BF16, tag="rl", bufs=1)
                nc.scalar.activation(out=rl, in_=accT, func=AF.Relu)
                elu = work.tile([C, N], BF16, tag="elu")
                nc.vector.affine_then_add(out=elu, in0=ex, in1=rl, scale=1.0, bias=-1.0)
                nc.sync.dma_start(out=ag_in[j], in_=elu)

            nc.gpsimd.collective_compute(
                "AllGather",
                mybir.AluOpType.bypass,
                replica_groups=[list(range(n_cores))],
                ins=[ag_in[:]],
                outs=[ag_out[:]],
            )
            if do_tail:
                for s in range(NSLICE):
                    a0 = work.tile([C, N], BF16, tag="ga0", bufs=2)
                    nc.sync.dma_start(out=a0, in_=ag_out[2 * s])
                    a1 = work.tile([C, N], BF16, tag="ga1", bufs=2)
                    nc.sync.dma_start(out=a1, in_=ag_out[2 * s + 1])
                    nc.vector.tensor_add(
                        out=avs[:, s % AVR, :], in0=a0, in1=a1
                    )
                    if debug and s < 6:
                        nc.sync.dma_start(out=dbg[8 + s, :C], in_=a0)
                    # run every tail slice whose inputs are now all cached;
                    # ring depth AVR covers the 3-slice window
                    for g in range(NTAIL):
                        b, t2 = divmod(g, T2)
                        if b * T1 + t2 + KT - 1 == s:
                            tail_slice(g)

    if not nc.is_finalized():
        nc.finalize()
    return nc


def _prep(inputs, n_cores):
    x = np.asarray(inputs["x"], np.float32)
    supports = np.asarray(inputs["supports"], np.float32)
    atten = np.asarray(inputs["atten_supports"], np.float32)
    w_t1 = np.asarray(inputs["w_t1"], np.float32)
    Wt = np.asarray(inputs["Wt"], np.float32)
    Wl = np.asarray(inputs["Wl"], np.float32)
    Wr = np.asarray(inputs["Wr"], np.float32)
    w_t2 = np.asarray(inputs["w_t2"], np.float32)
    gamma = np.asarray(inputs["gamma"], np.float32)
    beta = np.asarray(inputs["beta"], np.float32)

    fp8 = ml_dtypes.float8_e4m3fn
    bf16 = ml_dtypes.bfloat16
    xT = np.ascontiguousarray(x.transpose(0, 1, 3, 2)).astype(bf16)  # [B,T,C,N]
    w1T = np.ascontiguousarray(w_t1[:, :, :, 0].transpose(2, 1, 0)).astype(bf16)
    # conv(w2, 0.5*(h0+h1)) = conv(0.5*w2, h0+h1): head-mean folded into w2
    w2T = np.ascontiguousarray(
        (0.5 * w_t2[:, :, :, 0]).transpose(2, 1, 0)
    ).astype(bf16)
    mrel = (atten != 0).astype(np.float32)  # [R,N,N]
    msup = (supports != 0).astype(np.float32)  # [K1,N,N]
    mrelT = np.ascontiguousarray(mrel.transpose(0, 2, 1)).astype(fp8)
    msupT = np.ascontiguousarray(msup.transpose(0, 2, 1)).astype(fp8)
    uT = np.ascontiguousarray(
        np.minimum(mrel[0] + mrel[1] + msup, 1.0).transpose(0, 2, 1)
    ).astype(fp8)
    gbT = np.stack([gamma[0, 0].T, beta[0, 0].T]).astype(np.float32)  # [2,C,N]

    upc = NUNITS // n_cores
    in_maps = []
    for c in range(n_cores):
        xwa = np.empty((upc, KT, C, N), bf16)
        wcU = np.empty((upc, K1, C, C + 6), bf16)
        for j in range(upc):
            u = c * upc + j  # ag_out row index == global unit id
            sl, h = u // H, u % H
            b, t1 = sl // T1, sl % T1
            for tau in range(KT):
                xwa[j, tau] = xT[b, t1 + tau]
            for k in range(K1):
                wcU[j, k, :, :C] = Wt[h, k]
                wcU[j, k, :, C : C + 3] = Wt[h, k] @ Wr[h, k].T
                wcU[j, k, :, C + 3 :] = Wt[h, k] @ Wl[h, k].T
        in_maps.append(
            dict(xw=xwa, w1T=w1T, w2T=w2T, wcU=wcU,
                 mrelT=mrelT, msupT=msupT, uT=uT, gbT=gbT)
        )
    return in_maps


def kernel(**inputs):
    from concourse.bass_utils import run_bass_kernel_spmd

    if N_CORES not in _cache:
        _cache[N_CORES] = _build(N_CORES)
    nc = _cache[N_CORES]
    in_maps = _prep(inputs, N_CORES)
    res = run_bass_kernel_spmd(nc, in_maps, list(range(N_CORES)))
    o = np.asarray(res.results[0]["out"], np.float32)  # [B,T2,C,N]
    return np.ascontiguousarray(o.transpose(0, 1, 3, 2))


# revision 62
# speedup vs baseline: 1.2319x; 1.0112x over previous
"""STConvBlock Trainium2 kernel: tconv1(GLU) -> Cheb-attention -> tconv2(GLU) -> norm.

Sharding: 40 (slice, head) attention units, 5 per core (unit u = core*5 + j);
one AllGather of all unit outputs, then the replicated tail (tconv2+norm,
16 slices) runs from a ring cache of head-averaged slices. All adjacency
masks live in SBUF (fp8); all hot matmuls/DVE ops in bf16.

Attention math (per slice xs [N,C], head h, chebyshev k):
  wx = xs @ Wt;  al = xs @ (Wt@Wl.T);  ar = xs @ (Wt@Wr.T)   # [N,3]
  S[i,j] = sum_r m_r[i,j] * (al[i,r] + ar[j,r]); masked softmax_j; out += p @ wx
We build S TRANSPOSED (S^T[j,i]): ar[j,r] is a per-partition scalar, al is
broadcast via a K=1 PE matmul into PSUM, and each relation term is ONE fused
DVE scalar_tensor_tensor (al_bcast + ar) * mask. E^T = U*exp(S^T) feeds the
PE as rhs; lhsT=[wx|ones] yields p@wx (unscaled) and the softmax denominators
in one PSUM tile.
"""

import numpy as np
import ml_dtypes

B, T, N, C = 2, 12, 1024, 64
KT = 3
T1 = T - KT + 1   # 10
T2 = T1 - KT + 1  # 8
H, K1, R = 2, 3, 2
NSLICE = B * T1   # 20
NUNITS = NSLICE * H  # 40
N_CORES = 8
NT = N // 128  # 8
FCH = 512
NF = N // FCH  # 2
NC_ELEMS = float(N * C)
NTAIL = B * T2  # 16
# tail slices runnable after round j's AllGather (needs att slices <= 4j+3;
# tail g=(b,t2) needs slices b*T1+t2 .. +2)
TAIL_WINDOWS = [[0, 1], [2, 3, 4, 5], [6, 7], [8, 9, 10, 11], [12, 13, 14, 15]]
AVR = 6  # ring depth of cached head-averaged slices

_cache = {}


def _build(n_cores, do_att=True, do_tail=True, debug=False):
    import concourse.bass as bass
    import concourse.tile as tile
    import concourse.mybir as mybir
    from concourse import bacc
    from concourse.masks import make_identity

    F32 = mybir.dt.float32
    BF16 = mybir.dt.bfloat16
    FP8 = mybir.dt.float8e4
    AF = mybir.ActivationFunctionType
    ALU = mybir.AluOpType
    AX = mybir.AxisListType
    upc = NUNITS // n_cores

    nc = bacc.Bacc(None, target_bir_lowering=False)
    xw = nc.dram_tensor("xw", [upc, KT, C, N], BF16, kind="ExternalInput")
    w1T = nc.dram_tensor("w1T", [KT, C, 2 * C], BF16, kind="ExternalInput")
    w2T = nc.dram_tensor("w2T", [KT, C, 2 * C], BF16, kind="ExternalInput")
    # combined per-(unit,k) weights: cols [wt (C) | wr (3) | wl (3)]
    wcU = nc.dram_tensor("wcU", [upc, K1, C, C + 6], BF16, kind="ExternalInput")
    mrelT = nc.dram_tensor("mrelT", [R, N, N], FP8, kind="ExternalInput")
    msupT = nc.dram_tensor("msupT", [K1, N, N], FP8, kind="ExternalInput")
    uT = nc.dram_tensor("uT", [K1, N, N], FP8, kind="ExternalInput")
    gbT = nc.dram_tensor("gbT", [2, C, N], F32, kind="ExternalInput")
    out = nc.dram_tensor("out", [B, T2, C, N], F32, kind="ExternalOutput")
    dbg = (
        nc.dram_tensor("dbg", [14, 128, N], BF16, kind="ExternalOutput")
        if debug else None
    )
    ag_in = nc.dram_tensor("ag_in", [upc, C, N], BF16)
    ag_out = nc.dram_tensor("ag_out", [NUNITS, C, N], BF16, addr_space="Shared")

    with tile.TileContext(nc) as tc:
        with (
            tc.tile_pool(name="consts", bufs=1) as consts,
            tc.tile_pool(name="work", bufs=2) as work,
            tc.tile_pool(name="sc", bufs=2) as sc,
            tc.tile_pool(name="ps_al", bufs=1, space="PSUM") as ps_al,
            tc.tile_pool(name="ps_m", bufs=2, space="PSUM") as ps_m,
            tc.tile_pool(name="ps_big", bufs=1, space="PSUM") as ps_big,
            tc.tile_pool(name="ps_glu", bufs=1, space="PSUM") as ps_glu,
        ):
            # ---- resident constants ----
            mrel_sb = consts.tile([128, R, NT, N], FP8)
            nc.sync.dma_start(
                out=mrel_sb[:],
                in_=mrelT[:].rearrange("r (t p) n -> p r t n", p=128),
            )
            msup_sb = consts.tile([128, K1, NT, N], FP8)
            nc.sync.dma_start(
                out=msup_sb[:],
                in_=msupT[:].rearrange("k (t p) n -> p k t n", p=128),
            )
            w1_sb = consts.tile([C, KT, 2 * C], BF16)
            w2_sb = consts.tile([C, KT, 2 * C], BF16)
            nc.sync.dma_start(out=w1_sb[:], in_=w1T[:].rearrange("t c o -> c t o"))
            nc.sync.dma_start(out=w2_sb[:], in_=w2T[:].rearrange("t c o -> c t o"))
            wc_sb = consts.tile([C, upc, K1, C + 6], BF16)
            nc.sync.dma_start(out=wc_sb[:], in_=wcU[:].rearrange("j k c x -> c j k x"))
            gb_sb = consts.tile([C, 2, N], F32)
            nc.sync.dma_start(out=gb_sb[:, 0, :], in_=gbT[0])
            nc.sync.dma_start(out=gb_sb[:, 1, :], in_=gbT[1])
            ones1x128 = consts.tile([1, 128], BF16)
            nc.gpsimd.memset(ones1x128, 1.0)
            ones1x64 = consts.tile([1, C], F32)
            nc.gpsimd.memset(ones1x64, 1.0)
            ones64x1 = consts.tile([C, 1], F32)
            nc.gpsimd.memset(ones64x1, 1.0)
            id64 = consts.tile([C, C], F32)
            make_identity(nc, id64)
            eps_sb = consts.tile([1, 1], F32)
            nc.gpsimd.memset(eps_sb, 1e-6)
            # head-averaged attention outputs, ring-cached for the tail
            avs = consts.tile([C, AVR, N], BF16)

            def glu_conv(w_sb, rhs_tile, res_cb, out_tile, res_scale):
                """out[c, n] = (conv[0:C] + res_scale*res) * sigmoid(conv[C:2C])."""
                for f in range(NF):
                    cps = ps_glu.tile([2 * C, FCH], F32, tag="glu")
                    for tau in range(KT):
                        nc.tensor.matmul(
                            out=cps,
                            lhsT=w_sb[:, tau, :],
                            rhs=rhs_tile(tau)[:, f * FCH : (f + 1) * FCH],
                            start=(tau == 0),
                            stop=(tau == KT - 1),
                        )
                    sig = work.tile([C, FCH], BF16, tag="sig")
                    nc.scalar.activation(out=sig, in_=cps[C:, :], func=AF.Sigmoid)
                    po = work.tile([C, FCH], BF16, tag="po")
                    nc.vector.affine_then_add(
                        out=po, in0=res_cb(f), in1=cps[:C, :],
                        scale=res_scale, bias=0.0,
                    )
                    nc.vector.tensor_mul(
                        out=out_tile[:, f * FCH : (f + 1) * FCH], in0=po, in1=sig
                    )

            def tail_slice(g):
                b, t2 = divmod(g, T2)
                s0 = b * T1 + t2
                h2 = work.tile([C, N], F32, tag="h2", bufs=2)
                # w2 pre-scaled by 0.5 on host (head-mean fold); residual 0.5
                glu_conv(
                    w2_sb,
                    lambda tau: avs[:, (s0 + tau) % AVR, :],
                    lambda f: avs[:, (s0 + KT - 1) % AVR, f * FCH : (f + 1) * FCH],
                    h2,
                    0.5,
                )
                sums = work.tile([C, 1], F32, tag="sums")
                nc.vector.tensor_reduce(out=sums, in_=h2, axis=AX.X, op=ALU.add)
                sqs = work.tile([C, 1], F32, tag="sqs")
                scr = work.tile([C, N], F32, tag="scr", bufs=1)
                nc.scalar.activation(out=scr, in_=h2, func=AF.Square)
                nc.vector.tensor_reduce(out=sqs, in_=scr, axis=AX.X, op=ALU.add)
                pair = work.tile([C, 2], F32, tag="pair")
                nc.scalar.copy(out=pair[:, 0:1], in_=sums)
                nc.scalar.copy(out=pair[:, 1:2], in_=sqs)
                totp = ps_m.tile([1, 2], F32, tag="m")
                nc.tensor.matmul(
                    out=totp, lhsT=ones64x1, rhs=pair, start=True, stop=True
                )
                mu = work.tile([1, 1], F32, tag="mu")
                nc.scalar.mul(mu, totp[:, 0:1], 1.0 / NC_ELEMS)
                nmusq = work.tile([1, 1], F32, tag="nmusq")
                nc.vector.tensor_mul(out=nmusq, in0=mu, in1=mu)
                nc.scalar.mul(nmusq, nmusq, -1.0)
                var = work.tile([1, 1], F32, tag="var")
                nc.scalar.activation(
                    out=var, in_=totp[:, 1:2],
                    func=AF.Identity, scale=1.0 / NC_ELEMS, bias=nmusq,
                )
                sd = work.tile([1, 1], F32, tag="sd")
                nc.scalar.activation(out=sd, in_=var, func=AF.Sqrt, bias=eps_sb)
                rstd = work.tile([1, 1], F32, tag="rstd")
                nc.vector.reciprocal(out=rstd, in_=sd)
                nmr = work.tile([1, 1], F32, tag="nmr")
                nc.vector.tensor_mul(out=nmr, in0=mu, in1=rstd)
                nc.scalar.mul(nmr, nmr, -1.0)
                pr = work.tile([1, 2], F32, tag="pr")
                nc.scalar.copy(out=pr[:, 0:1], in_=rstd)
                nc.scalar.copy(out=pr[:, 1:2], in_=nmr)
                bcp = ps_m.tile([C, 2], F32, tag="m")
                nc.tensor.matmul(
                    out=bcp, lhsT=ones1x64, rhs=pr, start=True, stop=True
                )
                bc = work.tile([C, 2], F32, tag="bc")
                nc.scalar.copy(out=bc, in_=bcp)
                normed = work.tile([C, N], F32, tag="normed", bufs=1)
                nc.scalar.activation(
                    out=normed, in_=h2, func=AF.Identity,
                    scale=bc[:, 0:1], bias=bc[:, 1:2],
                )
                og = work.tile([C, N], F32, tag="og", bufs=1)
                nc.vector.tensor_mul(out=og, in0=normed, in1=gb_sb[:, 0, :])
                nc.vector.tensor_add(out=og, in0=og, in1=gb_sb[:, 1, :])
                nc.sync.dma_start(out=out[b, t2], in_=og)

            # ---- attention units (5 rounds, pipelined with tail) ----
            for j in range(upc if do_att else 0):
                xw_t = work.tile([C, KT, N], BF16, tag="xw", bufs=1)
                for tau in range(KT):
                    nc.sync.dma_start(out=xw_t[:, tau, :], in_=xw[j, tau])
                xsT = work.tile([C, N], BF16, tag="xsT")
                glu_conv(
                    w1_sb,
                    lambda tau: xw_t[:, tau, :],
                    lambda f: xw_t[:, KT - 1, f * FCH : (f + 1) * FCH],
                    xsT,
                    1.0,
                )
                accT = work.tile([C, N], BF16, tag="accT", bufs=1)
                for k in range(K1):
                    # per jt: one matmul -> [wx | ar] ; plus al rows
                    ars, wxos = [], []
                    for jt in range(NT):
                        cps = ps_m.tile([128, C + 3], F32, tag="m")
                        nc.tensor.matmul(
                            out=cps, lhsT=xsT[:, jt * 128 : (jt + 1) * 128],
                            rhs=wc_sb[:, j, k, : C + 3], start=True, stop=True,
                        )
                        ar_jt = work.tile([128, R + 1], F32, tag=f"ar{jt}", bufs=2)
                        nc.scalar.copy(out=ar_jt, in_=cps[:, C : C + 3])
                        ars.append(ar_jt)
                        wxo_jt = work.tile([128, C + 1], BF16, tag=f"wxo{jt}", bufs=2)
                        nc.gpsimd.memset(wxo_jt[:, C : C + 1], 1.0)
                        nc.scalar.copy(out=wxo_jt[:, :C], in_=cps[:, :C])
                        wxos.append(wxo_jt)
                    alTs = []
                    for r in range(R + 1):
                        alT_r = work.tile([1, N], BF16, tag=f"alT{r}", bufs=2)
                        alTs.append(alT_r)
                    for f in range(NF):
                        for r in range(R + 1):
                            alp = ps_m.tile([1, FCH], F32, tag="m")
                            nc.tensor.matmul(
                                out=alp,
                                lhsT=wc_sb[:, j, k, C + 3 + r : C + 4 + r],
                                rhs=xsT[:, f * FCH : (f + 1) * FCH],
                                start=True, stop=True,
                            )
                            nc.scalar.copy(
                                out=alTs[r][:, f * FCH : (f + 1) * FCH], in_=alp
                            )
                    # al broadcast -> SBUF bf16 [128, 3, N] (one ACT copy per f)
                    als_sb = work.tile([128, R + 1, N], BF16, tag="als")
                    for f in range(NF):
                        alp = ps_al.tile([128, R + 1, FCH], F32, tag="al")
                        for r in range(R + 1):
                            nc.tensor.matmul(
                                out=alp[:, r, :], lhsT=ones1x128,
                                rhs=alTs[r][:, f * FCH : (f + 1) * FCH],
                                start=True, stop=True,
                            )
                        nc.scalar.copy(
                            out=als_sb[:, :, f * FCH : (f + 1) * FCH], in_=alp
                        )
                    # union mask for this k: one batched DMA
                    uuk = sc.tile([128, NT, N], FP8, tag="uuk", bufs=1)
                    nc.sync.dma_start(
                        out=uuk, in_=uT[k].rearrange("(t p) n -> p t n", p=128)
                    )
                    opsf = []
                    for f in range(NF):
                        ops_f = ps_big.tile([C + 1, FCH], F32, tag=f"big{f}")
                        opsf.append(ops_f)
                    def emit_ee(eraw_p, jt_p):
                        # ee + its E-matmuls, deferred one tile so the DVE's
                        # strict FIFO never head-of-line blocks on ACT's exp
                        ee = sc.tile([128, N], BF16, tag="ee")
                        nc.vector.tensor_mul(
                            out=ee, in0=eraw_p, in1=uuk[:, jt_p, :]
                        )
                        for f in range(NF):
                            nc.tensor.matmul(
                                out=opsf[f], lhsT=wxos[jt_p],
                                rhs=ee[:, f * FCH : (f + 1) * FCH],
                                start=(jt_p == 0), stop=(jt_p == NT - 1),
                            )

                    pend = None
                    for jt in range(NT):
                        t0 = sc.tile([128, N], BF16, tag="t0")
                        nc.vector.scalar_tensor_tensor(
                            out=t0, in0=als_sb[:, 0, :], scalar=ars[jt][:, 0:1],
                            in1=mrel_sb[:, 0, jt, :],
                            op0=ALU.add, op1=ALU.mult,
                        )
                        t1 = sc.tile([128, N], BF16, tag="t1")
                        nc.vector.scalar_tensor_tensor(
                            out=t1, in0=als_sb[:, 1, :], scalar=ars[jt][:, 1:2],
                            in1=mrel_sb[:, 1, jt, :],
                            op0=ALU.add, op1=ALU.mult,
                        )
                        s01 = sc.tile([128, N], BF16, tag="s01")
                        nc.vector.tensor_add(out=s01, in0=t0, in1=t1)
                        t2 = sc.tile([128, N], BF16, tag="t2")
                        nc.vector.scalar_tensor_tensor(
                            out=t2, in0=als_sb[:, 2, :], scalar=ars[jt][:, 2:3],
                            in1=msup_sb[:, k, jt, :],
                            op0=ALU.add, op1=ALU.mult,
                        )
                        ss = sc.tile([128, N], BF16, tag="ss")
                        nc.vector.tensor_add(out=ss, in0=s01, in1=t2)
                        eraw = sc.tile([128, N], BF16, tag="eraw")
                        nc.scalar.activation(out=eraw, in_=ss, func=AF.Exp)
                        if pend is not None:
                            emit_ee(*pend)
                        pend = (eraw, jt)
                    emit_ee(*pend)
                    for f in range(NF):
                        fs = slice(f * FCH, (f + 1) * FCH)
                        ops = opsf[f]
                        # 1/den = exp(-ln(den)) on the ACT engine: frees the
                        # Vector engine from its slow iterative reciprocal
                        lnden = work.tile([1, FCH], F32, tag="lnden")
                        nc.scalar.activation(
                            out=lnden, in_=ops[C : C + 1, :], func=AF.Ln
                        )
                        recip = work.tile([1, FCH], F32, tag="recip")
                        nc.scalar.activation(
                            out=recip, in_=lnden, func=AF.Exp, scale=-1.0
                        )
                        rbp = ps_m.tile([C, FCH], F32, tag="m")
                        nc.tensor.matmul(
                            out=rbp, lhsT=ones1x64, rhs=recip, start=True, stop=True
                        )
                        rb_sb = work.tile([C, FCH], BF16, tag="rb")
                        nc.scalar.copy(out=rb_sb, in_=rbp)
                        pk = work.tile([C, FCH], BF16, tag="pk")
                        nc.scalar.copy(out=pk, in_=ops[:C, :])
                        if debug and j == 0 and k == 0 and f == 0:
                            nc.sync.dma_start(out=dbg[0, :C], in_=xsT)
                            nc.sync.dma_start(out=dbg[3, :C, :FCH], in_=pk)
                            nc.sync.dma_start(out=dbg[4, :C, :FCH], in_=rb_sb)
                            den = work.tile([1, FCH], BF16, tag="dbgden")
                            nc.scalar.copy(out=den, in_=ops[C : C + 1, :])
                            nc.sync.dma_start(out=dbg[5, 0:1, :FCH], in_=den)
                        if k == 0:
                            nc.vector.tensor_mul(
                                out=accT[:, fs], in0=pk, in1=rb_sb,
                            )
                        else:
                            tsc = work.tile([C, FCH], BF16, tag="tsc")
                            nc.vector.tensor_mul(out=tsc, in0=pk, in1=rb_sb)
                            nc.vector.tensor_add(
                                out=accT[:, fs], in0=accT[:, fs], in1=tsc,
                            )
                # elu(accT) = exp(min(a,0)) - 1 + relu(a)
                mn = work.tile([C, N], BF16, tag="mn", bufs=1)
                nc.vector.tensor_scalar_min(mn, accT, 0.0)
                ex = work.tile([C, N], BF16, tag="ex", bufs=1)
                nc.scalar.activation(out=ex, in_=mn, func=AF.Exp)
                rl = work.tile([C, N], BF16, tag="rl", bufs=1)
                nc.scalar.activation(out=rl, in_=accT, func=AF.Relu)
                elu = work.tile([C, N], BF16, tag="elu")
                nc.vector.affine_then_add(out=elu, in0=ex, in1=rl, scale=1.0, bias=-1.0)
                nc.sync.dma_start(out=ag_in[j], in_=elu)

            nc.gpsimd.collective_compute(
                "AllGather",
                mybir.AluOpType.bypass,
                replica_groups=[list(range(n_cores))],
                ins=[ag_in[:]],
                outs=[ag_out[:]],
            )
            if do_tail:
                for s in range(NSLICE):
                    a0 = work.tile([C, N], BF16, tag="ga0", bufs=2)
                    nc.sync.dma_start(out=a0, in_=ag_out[2 * s])
                    a1 = work.tile([C, N], BF16, tag="ga1", bufs=2)
                    nc.sync.dma_start(out=a1, in_=ag_out[2 * s + 1])
                    nc.vector.tensor_add(
                        out=avs[:, s % AVR, :], in0=a0, in1=a1
                    )
                    if debug and s < 6:
                        nc.sync.dma_start(out=dbg[8 + s, :C], in_=a0)
                    # run every tail slice whose inputs are now all cached;
                    # ring depth AVR covers the 3-slice window
                    for g in range(NTAIL):
                        b, t2 = divmod(g, T2)
                        if b * T1 + t2 + KT - 1 == s:
                            tail_slice(g)

    if not nc.is_finalized():
        nc.finalize()
    return nc


def _prep(inputs, n_cores):
    x = np.asarray(inputs["x"], np.float32)
    supports = np.asarray(inputs["supports"], np.float32)
    atten = np.asarray(inputs["atten_supports"], np.float32)
    w_t1 = np.asarray(inputs["w_t1"], np.float32)
    Wt = np.asarray(inputs["Wt"], np.float32)
    Wl = np.asarray(inputs["Wl"], np.float32)
    Wr = np.asarray(inputs["Wr"], np.float32)
    w_t2 = np.asarray(inputs["w_t2"], np.float32)
    gamma = np.asarray(inputs["gamma"], np.float32)
    beta = np.asarray(inputs["beta"], np.float32)

    fp8 = ml_dtypes.float8_e4m3fn
    bf16 = ml_dtypes.bfloat16
    xT = np.ascontiguousarray(x.transpose(0, 1, 3, 2)).astype(bf16)  # [B,T,C,N]
    w1T = np.ascontiguousarray(w_t1[:, :, :, 0].transpose(2, 1, 0)).astype(bf16)
    # conv(w2, 0.5*(h0+h1)) = conv(0.5*w2, h0+h1): head-mean folded into w2
    w2T = np.ascontiguousarray(
        (0.5 * w_t2[:, :, :, 0]).transpose(2, 1, 0)
    ).astype(bf16)
    mrel = (atten != 0).astype(np.float32)  # [R,N,N]
    msup = (supports != 0).astype(np.float32)  # [K1,N,N]
    mrelT = np.ascontiguousarray(mrel.transpose(0, 2, 1)).astype(fp8)
    msupT = np.ascontiguousarray(msup.transpose(0, 2, 1)).astype(fp8)
    uT = np.ascontiguousarray(
        np.minimum(mrel[0] + mrel[1] + msup, 1.0).transpose(0, 2, 1)
    ).astype(fp8)
    gbT = np.stack([gamma[0, 0].T, beta[0, 0].T]).astype(np.float32)  # [2,C,N]

    upc = NUNITS // n_cores
    in_maps = []
    for c in range(n_cores):
        xwa = np.empty((upc, KT, C, N), bf16)
        wcU = np.empty((upc, K1, C, C + 6), bf16)
        for j in range(upc):
            u = c * upc + j  # ag_out row index == global unit id
            sl, h = u // H, u % H
            b, t1 = sl // T1, sl % T1
            for tau in range(KT):
                xwa[j, tau] = xT[b, t1 + tau]
            for k in range(K1):
                wcU[j, k, :, :C] = Wt[h, k]
                wcU[j, k, :, C : C + 3] = Wt[h, k] @ Wr[h, k].T
                wcU[j, k, :, C + 3 :] = Wt[h, k] @ Wl[h, k].T
        in_maps.append(
            dict(xw=xwa, w1T=w1T, w2T=w2T, wcU=wcU,
                 mrelT=mrelT, msupT=msupT, uT=uT, gbT=gbT)
        )
    return in_maps


def kernel(**inputs):
    from concourse.bass_utils import run_bass_kernel_spmd

    if N_CORES not in _cache:
        _cache[N_CORES] = _build(N_CORES)
    nc = _cache[N_CORES]
    in_maps = _prep(inputs, N_CORES)
    res = run_bass_kernel_spmd(nc, in_maps, list(range(N_CORES)))
    o = np.asarray(res.results[0]["out"], np.float32)  # [B,T2,C,N]
    return np.ascontiguousarray(o.transpose(0, 1, 3, 2))
